# revision 10
# baseline (speedup 1.0000x reference)
"""Trainium2 Bass kernel v3 for spatial self-attention (nn_Attention_90615220011343).

Per-core (core c -> batch c//2, heads 2*(c%2), 2*(c%2)+1):
    qkv = x @ w_qkv; per head sim^T[j,i] = k^T q; attn = softmax; out = attn@v
    y_partial = sum_h (out_h/den) @ wo_h ; host sums head-pairs + bias.

v3 changes vs v2 (cost-model driven):
  - sim matmul in fp8e4m3 with MatmulPerfMode.DoubleRow: 0.5 cycles/row
    (vs 1.0 for bf16/fp32r) -> 256 PE cycles per [128,512] chunk.
    Precision recovered by error compensation: q = qhi + qlo, k = khi + klo
    (hi = fp8(x), lo = fp8(x - hi)); the 128 DoubleRow contraction slots
    (64 partitions x 2) hold all four cross products (qhi+qlo)x(khi+klo),
    so the product is exact up to the ~0.1% lo-rounding. attn scale (1/sqrt(32))
    is folded into the exp input scaling, not into q.
  - exp runs on THREE engines: ACT (exact, activation Exp with scale),
    DVE + Pool (Schraudolph int16 bitcast = bf16 exp approx). Pattern
    weighted by engine rates (ACT 0.83, DVE 1.04, Pool 1.39 ns/row).
  - q^T/k^T builds write 2-itile/panel stacks into one [128,512] psum
    ([64*s + 32*h + d] partitions), so the fp8 hi-copy + lo-subtract are
    [128,512] ops (4x fewer engine rows); SBUF->SBUF DMAs (cheap issue from
    the gpsimd ring) fold the stacks into the DoubleRow operand layouts:
      qSide[h]: [64, 2, N] rows = (qhi d | qlo d), t duplicated
      kSide[h]: [64, 2, N] cols t = (khi | klo), rows duplicated
  - transposes of the normalized attention output are bf16 (1 c/r, vs fp32r
    4 c/r when free<256); outT copy reads bf16 psum (DVE 2x_1p mode).
"""

import numpy as np

HEADS = 4
DH = 32
N = 4096
C = 256
P = 128
NCH = 32          # j-chunks of 128 tokens
ITILES = 8        # i tiles of 512
ROT = 6           # rotating psum banks for sim chunks
EROT = 12         # eslab rotation depth (chunks)
AV_LAG = 10       # chunks between sim emission and its av matmuls
SCALE = float(DH ** -0.5)
# bf16 Schraudolph exp: round(s*a+b) as int16 IS bf16(exp(s)) up to ~3%
# sawtooth; softmax normalization cancels most. a absorbs the attn scale.
SCH_A = float(2 ** 7 / np.log(2)) * SCALE
SCH_B = float(127 * 2 ** 7) - 7.6
# exp engine pattern per chunk (1 quantum = 1 chunk = [128,512] psum slot):
# weighted by measured engine service times (ACT 612, DVE 658, Pool ~560 ns)
EXP_W = {"A": 11, "V": 10, "P": 12}

_CACHED = {}


def _make_pattern(total, weights):
    acc = {k: 0.0 for k in weights}
    wsum = float(sum(weights.values()))
    out = []
    for _ in range(total):
        for k in acc:
            acc[k] += weights[k]
        kbest = max(acc, key=lambda kk: (acc[kk], kk))
        acc[kbest] -= wsum
        out.append(kbest)
    return out


def _build_nc():
    import concourse.bass as bass
    import concourse.mybir as mybir
    from concourse.tile import TileContext
    from concourse.masks import make_identity

    FP = mybir.dt.float32
    BF = mybir.dt.bfloat16
    E4 = mybir.dt.float8e4
    U16 = mybir.dt.uint16
    I16 = mybir.dt.int16
    AF = mybir.ActivationFunctionType
    ALU = mybir.AluOpType
    DR = mybir.MatmulPerfMode.DoubleRow

    nc = bass.Bass(target_bir_lowering=False)
    xt_d = nc.declare_dram_parameter("xt", [C, N], U16, isOutput=False)
    wq_d = nc.declare_dram_parameter("wq", [C, 64], FP, isOutput=False)
    wk_d = nc.declare_dram_parameter("wk", [C, 64], FP, isOutput=False)
    wv_d = nc.declare_dram_parameter("wv", [C, 64], FP, isOutput=False)
    wo_d = nc.declare_dram_parameter("wo", [64, C], FP, isOutput=False)
    y_d = nc.declare_dram_parameter("y", [N, C], FP, isOutput=True)

    with TileContext(nc) as tc:
        with (
            tc.tile_pool(name="const", bufs=1) as constp,
            tc.tile_pool(name="big", bufs=1) as bigp,
            tc.tile_pool(name="ytmp", bufs=4) as ytmpp,
            tc.tile_pool(name="psR", bufs=1, space="PSUM") as psR,
            tc.tile_pool(name="psV", bufs=1, space="PSUM") as psV,
            tc.tile_pool(name="psT", bufs=1, space="PSUM") as psT,
        ):
            ident = constp.tile([P, P], FP, tag="ident")
            make_identity(nc, ident[:])
            identb = constp.tile([P, P], BF, tag="identb")
            nc.vector.tensor_copy(out=identb[:], in_=ident[:])

            # ---- persistent SBUF ----
            xT = [bigp.tile([P, N], BF, tag=f"xT{cc}", name=f"xT{cc}") for cc in range(2)]
            qSide = [bigp.tile([64, 2, N], E4, tag=f"qS{h}", name=f"qS{h}")
                     for h in range(2)]
            kSide = [bigp.tile([64, 2, N], E4, tag=f"kS{h}", name=f"kS{h}")
                     for h in range(2)]
            vaug = [bigp.tile([P, 33 * NCH], BF, tag=f"vaug{h}", name=f"vaug{h}")
                    for h in range(2)]
            outT = bigp.tile([64, N], BF, tag="outT")
            rden = bigp.tile([P, 64], FP, tag="rden")
            av_sc = bigp.tile([P, P], BF, tag="av_sc")
            eslabs = [bigp.tile([P, 1024], BF, tag=f"esl{t}", name=f"esl{t}")
                      for t in range(EROT // 2)]

            wq_sb = bigp.tile([P, 2, 64], BF, tag="wq")
            wk_sb = bigp.tile([P, 2, 64], BF, tag="wk")
            wv_sb = bigp.tile([P, 2, 64], BF, tag="wv")
            wo_sb = bigp.tile([64, C], BF, tag="wo")

            # ---- psum ----
            rots = [psR.tile([P, 1024], FP, tag=f"R{t}", name=f"rotT{t}")
                    for t in range(ROT // 2)]
            avp = psV.tile([P, 512], FP, tag="V")      # cols 0:132 in use
            tb = psT.tile([P, 512], FP, tag="T")       # y projections

            def rhalf(bc):
                return rots[(bc % ROT) // 2], 512 * (bc % 2)

            # ---- weight loads + conversion ----
            wq_st = bigp.tile([P, 2, 64], FP, tag="wq_st")
            wk_st = bigp.tile([P, 2, 64], FP, tag="wk_st")
            wv_st = bigp.tile([P, 2, 64], FP, tag="wv_st")
            wo_st = bigp.tile([64, C], FP, tag="wo_st")
            for cc in range(2):
                nc.sync.dma_start(out=wq_st[:, cc, :], in_=wq_d[cc * P:(cc + 1) * P, :])
                nc.sync.dma_start(out=wk_st[:, cc, :], in_=wk_d[cc * P:(cc + 1) * P, :])
                nc.sync.dma_start(out=wv_st[:, cc, :], in_=wv_d[cc * P:(cc + 1) * P, :])
            nc.sync.dma_start(out=wo_st[:], in_=wo_d[:])
            nc.vector.tensor_copy(out=wq_sb[:], in_=wq_st[:])
            nc.vector.tensor_copy(out=wk_sb[:], in_=wk_st[:])
            nc.vector.tensor_copy(out=wv_sb[:], in_=wv_st[:])
            nc.vector.tensor_copy(out=wo_sb[:], in_=wo_st[:])

            # ---- x load (pre-transposed bf16 from host), 3 DMA rings ----
            dma_engines = [nc.sync, nc.scalar, nc.gpsimd]
            for s in range(4):
                for cc in range(2):
                    dma_engines[(2 * s + cc) % 3].dma_start(
                        out=xT[cc][:, 1024 * s:1024 * (s + 1)].bitcast(U16),
                        in_=xt_d[P * cc:P * (cc + 1),
                                 1024 * s:1024 * (s + 1)],
                    )

            ones_st = bigp.tile([P, NCH], BF, tag="ones_st")
            nc.gpsimd.memset(ones_st[:], 1.0)
            for h in range(2):
                vv = vaug[h][:].rearrange("p (k e) -> p k e", e=33)
                nc.vector.tensor_copy(out=vv[:, :, 32], in_=ones_st[:])

            bankc = 0  # global rotating-slot cursor

            # ---- qkv builds -------------------------------------------------
            # q/k stage g covers FOUR itiles (4g..4g+4) stacked in one
            # [128,1024] psum pair: rows 64*s + 32*h + d, cols 512*c2 hold
            # itile 4g+2s+c2.  Then per (s,h): one [32,1024] fp8 hi-copy and
            # one [32,1024] lo-subtract straight into the DoubleRow operand
            # tiles (partition-shifted engine ops; no staging, no fold DMAs):
            #   qSide[h]: rows 0:32 = hi, 32:64 = lo; t dim duplicated by DMA
            #   kSide[h]: t=0 = hi, t=1 = lo; rows 32:64 duplicated by DMA
            cpeng = [nc.scalar, nc.vector, nc.scalar, nc.gpsimd]
            sbeng = [nc.vector, nc.gpsimd, nc.vector, nc.vector]

            def qk_stage(w_sb, side, g):
                nonlocal bankc
                assert bankc % 2 == 0, bankc
                rt, _ = rhalf(bankc)
                bankc += 2
                for s in range(2):
                    for c2 in range(2):
                        it = 4 * g + 2 * s + c2
                        for cc in range(2):
                            nc.tensor.matmul(
                                rt[64 * s:64 * (s + 1), 512 * c2:512 * (c2 + 1)],
                                lhsT=w_sb[:, cc, :],
                                rhs=xT[cc][:, 512 * it:512 * (it + 1)],
                                start=(cc == 0), stop=(cc == 1),
                                tile_position=(0, 64 * s),
                                skip_group_check=True,
                            )
                for s in range(2):
                    cols = slice(1024 * (2 * g + s), 1024 * (2 * g + s) + 1024)
                    for h in range(2):
                        r0 = 64 * s + 32 * h
                        if side is qSide:
                            hi_ap = side[h][0:32, 0, cols]
                            lo_ap = side[h][32:64, 0, cols]
                        else:
                            hi_ap = side[h][0:32, 0, cols]
                            lo_ap = side[h][0:32, 1, cols]
                        eng = cpeng[2 * s + h]
                        if eng is nc.scalar:
                            eng.copy(out=hi_ap, in_=rt[r0:r0 + 32, :])
                        else:
                            eng.tensor_copy(out=hi_ap, in_=rt[r0:r0 + 32, :])
                        sbeng[2 * s + h].tensor_tensor(
                            out=lo_ap, in0=rt[r0:r0 + 32, :], in1=hi_ap,
                            op=ALU.subtract,
                        )

            def dup_group(g):
                # duplicate this 2048-col group's qSide t dim / kSide rows
                cols = slice(2048 * g, 2048 * (g + 1))
                for h in range(2):
                    nc.sync.dma_start(out=qSide[h][:, 1, cols],
                                      in_=qSide[h][:, 0, cols])
                    nc.sync.dma_start(out=kSide[h][32:64, :, cols],
                                      in_=kSide[h][0:32, :, cols])

            def v_round(k0):
                nonlocal bankc
                rt2, c02 = rhalf(bankc)
                bankc += 1
                for k in range(k0, k0 + 4):
                    for cc in range(2):
                        nc.tensor.matmul(
                            rt2[:, c02 + 64 * (k - k0):
                                c02 + 64 * (k - k0) + 64],
                            lhsT=xT[cc][:, P * k:P * (k + 1)],
                            rhs=wv_sb[:, cc, :],
                            start=(cc == 0), stop=(cc == 1),
                        )
                sv2 = rt2[:, c02: c02 + 256].rearrange("p (k d) -> p k d", d=64)
                for h in range(2):
                    vv = vaug[h][:].rearrange("p (k e) -> p k e", e=33)
                    nc.vector.tensor_copy(
                        out=vv[:, k0:k0 + 4, 0:32],
                        in_=sv2[:, :, 32 * h:32 * (h + 1)],
                    )

            # prologue: all of q/k/v for BOTH heads (x DMA pieces feed in
            # column order; stage g needs cols 2048g:2048(g+1))
            for g in range(2):
                qk_stage(wk_sb, kSide, g)
                v_round(16 * g)
                v_round(16 * g + 4)
                qk_stage(wq_sb, qSide, g)
                v_round(16 * g + 8)
                v_round(16 * g + 12)
                dup_group(g)

            # ================= attention chunk stream ======================
            def chunk_meta(c):
                h = c // (ITILES * NCH)
                it = (c // NCH) % ITILES
                j = c % NCH
                return h, it, j

            NC_TOT = 2 * ITILES * NCH
            pattern = _make_pattern(NC_TOT, EXP_W)

            slot_of = {}

            def emit_sim(c):
                nonlocal bankc
                h, it, j = chunk_meta(c)
                slot_of[c] = bankc
                rt_, c0 = rhalf(bankc)
                bankc += 1
                nc.tensor.matmul(
                    rt_[:, c0:c0 + 512],
                    lhsT=kSide[h][:, :, P * j:P * (j + 1)],
                    rhs=qSide[h][:, :, 512 * it:512 * (it + 1)],
                    start=True, stop=True,
                    perf_mode=DR,
                )

            def emit_exp(c):
                # quantum = one chunk -> one rot half-slot, one eslab half
                s0 = slot_of[c]
                rt_, c0 = rhalf(s0)
                es = eslabs[(c % EROT) // 2]
                e0 = 512 * (c % 2)
                es_ap = es[:, e0:e0 + 512]
                rt_ap = rt_[:, c0:c0 + 512]
                eng = pattern[c % len(pattern)]
                if eng == "A":
                    nc.scalar.activation(es_ap, rt_ap, AF.Exp, scale=SCALE)
                elif eng == "V":
                    nc.vector.tensor_scalar(
                        out=es_ap.bitcast(I16), in0=rt_ap,
                        scalar1=SCH_A, scalar2=SCH_B,
                        op0=ALU.mult, op1=ALU.add,
                    )
                else:
                    nc.gpsimd.tensor_scalar(
                        out=es_ap.bitcast(I16), in0=rt_ap,
                        scalar1=SCH_A, scalar2=SCH_B,
                        op0=ALU.mult, op1=ALU.add,
                    )

            def emit_av(c):
                h, it, j = chunk_meta(c)
                es = eslabs[(c % EROT) // 2]
                e0 = 512 * (c % 2)
                for ic in range(4):
                    nc.tensor.matmul(
                        avp[:, 33 * ic:33 * ic + 33],
                        lhsT=es[:, e0 + 128 * ic:e0 + 128 * (ic + 1)],
                        rhs=vaug[h][:, 33 * j:33 * j + 33],
                        start=(j == 0 and ic == 0), stop=(j == NCH - 1),
                        skip_group_check=True,
                    )

            def emit_itile_stage(h, it):
                dv = avp[:, 0:132].rearrange("p (ic e) -> p ic e", e=33)[:, :, 32]
                r0 = 32 * h + 4 * it
                nc.vector.reciprocal(out=rden[:, r0:r0 + 4], in_=dv)
                for ic in range(4):
                    nc.gpsimd.tensor_scalar_mul(
                        av_sc[:, 32 * ic:32 * (ic + 1)],
                        avp[:, 33 * ic:33 * ic + 32],
                        rden[:, r0 + ic:r0 + ic + 1],
                    )

            def emit_itile_transpose(h, it):
                nonlocal bankc
                rt_, c0 = rhalf(bankc)
                bankc += 1
                for ic in range(4):
                    nc.tensor.transpose(
                        rt_[0:32, c0 + 64 * ic:c0 + 64 * (ic + 1)].bitcast(BF),
                        av_sc[:, 32 * ic:32 * (ic + 1)],
                        identb[:],
                    )
                nc.vector.tensor_copy(
                    out=outT[32 * h:32 * (h + 1), 512 * it:512 * (it + 1)],
                    in_=rt_[0:32, c0:c0 + 256].bitcast(BF),
                )

            def emit_y(it):
                for ic in range(4):
                    k = 4 * it + ic
                    cols = slice(256 * (ic % 2), 256 * (ic % 2) + C)
                    nc.tensor.matmul(
                        tb[:, cols],
                        lhsT=outT[:, P * k:P * (k + 1)],
                        rhs=wo_sb[:],
                        start=True, stop=True,
                        tile_position=(0, 0),
                    )
                    yo = ytmpp.tile([P, C], FP, tag="yo")
                    nc.scalar.copy(out=yo[:], in_=tb[:, cols])
                    nc.sync.dma_start(out=y_d[P * k:P * (k + 1), :], in_=yo[:])

            hooks = {}

            def add_hook(c, fn):
                hooks.setdefault(min(c, NC_TOT - 1), []).append(fn)

            for c in range(NC_TOT):
                emit_sim(c)
                emit_exp(c)
                if c >= AV_LAG:
                    # av of c-AV_LAG MUST precede the exp quantum closing at c:
                    # that exp overwrites the eslab cols av(c-AV_LAG) reads.
                    emit_av(c - AV_LAG)
                    ch, cit, cj = chunk_meta(c - AV_LAG)
                    if cj == NCH - 1:
                        emit_itile_stage(ch, cit)
                        add_hook(c + 10, lambda ch=ch, cit=cit:
                                 emit_itile_transpose(ch, cit))
                        if ch == 1 and cit > 0:
                            add_hook(c + 14, lambda cit=cit: emit_y(cit - 1))
                for fn in hooks.pop(c, ()):
                    fn()

            # tail: remaining avs, last i-tile stage/transpose, last y projs
            for c in range(NC_TOT - AV_LAG, NC_TOT):
                emit_av(c)
                ch, cit, cj = chunk_meta(c)
                if cj == NCH - 1:
                    emit_itile_stage(ch, cit)
                    emit_itile_transpose(ch, cit)
            for fn_list in [hooks[k] for k in sorted(hooks)]:
                for fn in fn_list:
                    fn()
            emit_y(ITILES - 2)
            emit_y(ITILES - 1)

    _split_excess_waits(nc, mybir)
    return nc


def _split_excess_waits(nc, mybir, maxw=1, carrier_cap=1):
    """Hoist excess semaphore waits onto InstEventSemaphore carriers."""
    skip = {
        "InstEventSemaphore", "InstCall",
        "InstUnconditionalBranch", "InstISA", "InstRegisterMove",
    }
    for f in nc.m.functions:
        for blk in f.blocks:
            idx = 0
            while idx < len(blk.instructions):
                ins = blk.instructions[idx]
                si = getattr(ins, "sync_info", None)
                if (
                    si is not None and si.on_wait and len(si.on_wait) > maxw
                    and type(ins).__name__ not in skip
                ):
                    waits = list(si.on_wait)
                    keep, excess = waits[:maxw], waits[maxw:]
                    at = idx
                    if (at > 0 and type(blk.instructions[at - 1]).__name__
                            == "InstLdweights"):
                        at -= 1
                    n_ins = 0
                    for i in range(0, len(excess), carrier_cap):
                        ev = mybir.InstEventSemaphore(
                            name=nc.get_next_instruction_name(),
                            engine=ins.engine,
                            ins=[], outs=[],
                            sync_info=mybir.SyncInfo(
                                on_wait=excess[i:i + carrier_cap], on_update=[]
                            ),
                        )
                        nc.register_instruction(ev)
                        blk.instructions.insert(at + n_ins, ev)
                        n_ins += 1
                    ins.sync_info = mybir.SyncInfo(
                        on_wait=keep, on_update=list(si.on_update or [])
                    )
                    idx += n_ins
                idx += 1
    return nc


def get_nc():
    if "nc" not in _CACHED:
        _CACHED["nc"] = _build_nc()
    return _CACHED["nc"]


def make_in_maps(x, w_qkv, w_out):
    """Host-side sharding: core c -> batch c//2, heads (c%2)*2, (c%2)*2+1."""
    import ml_dtypes
    B = x.shape[0]
    xf = np.ascontiguousarray(x.reshape(B, N, C))
    in_maps = []
    for core in range(8):
        b, hp = core // 2, core % 2
        h0, h1 = 2 * hp, 2 * hp + 1
        wq = np.concatenate(
            [w_qkv[:, h * DH:(h + 1) * DH] for h in (h0, h1)], axis=1
        )
        wk = np.concatenate(
            [w_qkv[:, 128 + h * DH: 128 + (h + 1) * DH] for h in (h0, h1)], axis=1
        )
        wv = np.concatenate(
            [w_qkv[:, 256 + h * DH: 256 + (h + 1) * DH] for h in (h0, h1)], axis=1
        )
        wo = np.concatenate(
            [w_out[h * DH:(h + 1) * DH, :] for h in (h0, h1)], axis=0
        )
        in_maps.append({
            "xt": np.ascontiguousarray(xf[b].T.astype(ml_dtypes.bfloat16)).view(np.uint16),
            "wq": np.ascontiguousarray(wq.astype(np.float32)),
            "wk": np.ascontiguousarray(wk.astype(np.float32)),
            "wv": np.ascontiguousarray(wv.astype(np.float32)),
            "wo": np.ascontiguousarray(wo.astype(np.float32)),
        })
    return in_maps


def kernel(x, w_qkv, w_out, b_out):
    from concourse.bass_utils import run_bass_kernel_spmd

    nc = get_nc()
    in_maps = make_in_maps(
        np.asarray(x, dtype=np.float32),
        np.asarray(w_qkv, dtype=np.float32),
        np.asarray(w_out, dtype=np.float32),
    )
    res = run_bass_kernel_spmd(nc, in_maps, list(range(8))).results
    B, H, W = 4, 64, 64
    y = np.empty((B, N, C), dtype=np.float32)
    for b in range(B):
        y[b] = res[2 * b]["y"] + res[2 * b + 1]["y"]
    y += np.asarray(b_out, dtype=np.float32)
    return y.reshape(B, H, W, C)


# revision 11
# speedup vs baseline: 1.3132x; 1.3132x over previous
"""Trainium2 Bass kernel v3 for spatial self-attention (nn_Attention_90615220011343).

Per-core (core c -> batch c//2, heads 2*(c%2), 2*(c%2)+1):
    qkv = x @ w_qkv; per head sim^T[j,i] = k^T q; attn = softmax; out = attn@v
    y_partial = sum_h (out_h/den) @ wo_h ; host sums head-pairs + bias.

v3 changes vs v2 (cost-model driven):
  - sim matmul in fp8e4m3 with MatmulPerfMode.DoubleRow: 0.5 cycles/row
    (vs 1.0 for bf16/fp32r) -> 256 PE cycles per [128,512] chunk.
    Precision recovered by error compensation: q = qhi + qlo, k = khi + klo
    (hi = fp8(x), lo = fp8(x - hi)); the 128 DoubleRow contraction slots
    (64 partitions x 2) hold all four cross products (qhi+qlo)x(khi+klo),
    so the product is exact up to the ~0.1% lo-rounding. attn scale (1/sqrt(32))
    is folded into the exp input scaling, not into q.
  - exp runs on THREE engines: ACT (exact, activation Exp with scale),
    DVE + Pool (Schraudolph int16 bitcast = bf16 exp approx). Pattern
    weighted by engine rates (ACT 0.83, DVE 1.04, Pool 1.39 ns/row).
  - q^T/k^T builds write 2-itile/panel stacks into one [128,512] psum
    ([64*s + 32*h + d] partitions), so the fp8 hi-copy + lo-subtract are
    [128,512] ops (4x fewer engine rows); SBUF->SBUF DMAs (cheap issue from
    the gpsimd ring) fold the stacks into the DoubleRow operand layouts:
      qSide[h]: [64, 2, N] rows = (qhi d | qlo d), t duplicated
      kSide[h]: [64, 2, N] cols t = (khi | klo), rows duplicated
  - transposes of the normalized attention output are bf16 (1 c/r, vs fp32r
    4 c/r when free<256); outT copy reads bf16 psum (DVE 2x_1p mode).
"""

import numpy as np

HEADS = 4
DH = 32
N = 4096
C = 256
P = 128
NCH = 32          # j-chunks of 128 tokens
ITILES = 8        # i tiles of 512
ROT = 6           # rotating psum banks for sim chunks
EROT = 12         # eslab rotation depth (chunks)
AV_LAG = 10       # chunks between sim emission and its av matmuls
SCALE = float(DH ** -0.5)
# bf16 Schraudolph exp: round(s*a+b) as int16 IS bf16(exp(s)) up to ~3%
# sawtooth; softmax normalization cancels most. a absorbs the attn scale.
SCH_A = float(2 ** 7 / np.log(2)) * SCALE
SCH_B = float(127 * 2 ** 7) - 7.6
# exp engine pattern per chunk (1 quantum = 1 chunk = [128,512] psum slot):
# weighted by measured engine service times (ACT 612, DVE 658, Pool ~560 ns)
EXP_W = {"A": 11, "V": 10, "P": 12}

_CACHED = {}


def _make_pattern(total, weights):
    acc = {k: 0.0 for k in weights}
    wsum = float(sum(weights.values()))
    out = []
    for _ in range(total):
        for k in acc:
            acc[k] += weights[k]
        kbest = max(acc, key=lambda kk: (acc[kk], kk))
        acc[kbest] -= wsum
        out.append(kbest)
    return out


def _build_nc():
    import concourse.bass as bass
    import concourse.mybir as mybir
    from concourse.tile import TileContext
    from concourse.masks import make_identity

    FP = mybir.dt.float32
    BF = mybir.dt.bfloat16
    E4 = mybir.dt.float8e4
    U16 = mybir.dt.uint16
    I16 = mybir.dt.int16
    AF = mybir.ActivationFunctionType
    ALU = mybir.AluOpType
    DR = mybir.MatmulPerfMode.DoubleRow

    nc = bass.Bass(target_bir_lowering=False)
    xt_d = nc.declare_dram_parameter("xt", [C, N], U16, isOutput=False)
    wq_d = nc.declare_dram_parameter("wq", [C, 64], FP, isOutput=False)
    wk_d = nc.declare_dram_parameter("wk", [C, 64], FP, isOutput=False)
    wv_d = nc.declare_dram_parameter("wv", [C, 64], FP, isOutput=False)
    wo_d = nc.declare_dram_parameter("wo", [64, C], FP, isOutput=False)
    y_d = nc.declare_dram_parameter("y", [N, C], FP, isOutput=True)

    with TileContext(nc) as tc:
        with (
            tc.tile_pool(name="const", bufs=1) as constp,
            tc.tile_pool(name="big", bufs=1) as bigp,
            tc.tile_pool(name="ytmp", bufs=4) as ytmpp,
            tc.tile_pool(name="psR", bufs=1, space="PSUM") as psR,
            tc.tile_pool(name="psV", bufs=1, space="PSUM") as psV,
            tc.tile_pool(name="psT", bufs=1, space="PSUM") as psT,
        ):
            ident = constp.tile([P, P], FP, tag="ident")
            make_identity(nc, ident[:])
            identb = constp.tile([P, P], BF, tag="identb")
            nc.vector.tensor_copy(out=identb[:], in_=ident[:])

            # ---- persistent SBUF ----
            xT = [bigp.tile([P, N], BF, tag=f"xT{cc}", name=f"xT{cc}") for cc in range(2)]
            qSide = [bigp.tile([64, 2, N], E4, tag=f"qS{h}", name=f"qS{h}")
                     for h in range(2)]
            kSide = [bigp.tile([64, 2, N], E4, tag=f"kS{h}", name=f"kS{h}")
                     for h in range(2)]
            vaug = [bigp.tile([P, 33 * NCH], BF, tag=f"vaug{h}", name=f"vaug{h}")
                    for h in range(2)]
            outT = bigp.tile([64, N], BF, tag="outT")
            rden = bigp.tile([P, 64], FP, tag="rden")
            av_sc = bigp.tile([P, P], BF, tag="av_sc")
            eslabs = [bigp.tile([P, 512], BF, tag=f"esl{t}", name=f"esl{t}")
                      for t in range(EROT)]

            wq_sb = bigp.tile([P, 2, 64], BF, tag="wq")
            wk_sb = bigp.tile([P, 2, 64], BF, tag="wk")
            wv_sb = bigp.tile([P, 2, 64], BF, tag="wv")
            wo_sb = bigp.tile([64, C], BF, tag="wo")

            # ---- psum ----
            rots = [psR.tile([P, 512], FP, tag=f"R{t}", name=f"rotT{t}")
                    for t in range(ROT)]
            avp = psV.tile([P, 512], FP, tag="V")      # cols 0:132 in use
            tb = psT.tile([P, 512], FP, tag="T")       # y projections

            def rhalf(bc):
                return rots[bc % ROT], 0

            # ---- weight loads + conversion ----
            wq_st = bigp.tile([P, 2, 64], FP, tag="wq_st")
            wk_st = bigp.tile([P, 2, 64], FP, tag="wk_st")
            wv_st = bigp.tile([P, 2, 64], FP, tag="wv_st")
            wo_st = bigp.tile([64, C], FP, tag="wo_st")
            for cc in range(2):
                nc.sync.dma_start(out=wq_st[:, cc, :], in_=wq_d[cc * P:(cc + 1) * P, :])
                nc.sync.dma_start(out=wk_st[:, cc, :], in_=wk_d[cc * P:(cc + 1) * P, :])
                nc.sync.dma_start(out=wv_st[:, cc, :], in_=wv_d[cc * P:(cc + 1) * P, :])
            nc.sync.dma_start(out=wo_st[:], in_=wo_d[:])
            nc.vector.tensor_copy(out=wq_sb[:], in_=wq_st[:])
            nc.vector.tensor_copy(out=wk_sb[:], in_=wk_st[:])
            nc.vector.tensor_copy(out=wv_sb[:], in_=wv_st[:])
            nc.vector.tensor_copy(out=wo_sb[:], in_=wo_st[:])

            # ---- x load (pre-transposed bf16 from host), 3 DMA rings ----
            dma_engines = [nc.sync, nc.scalar, nc.gpsimd]
            for s in range(4):
                for cc in range(2):
                    dma_engines[(2 * s + cc) % 3].dma_start(
                        out=xT[cc][:, 1024 * s:1024 * (s + 1)].bitcast(U16),
                        in_=xt_d[P * cc:P * (cc + 1),
                                 1024 * s:1024 * (s + 1)],
                    )

            ones_st = bigp.tile([P, NCH], BF, tag="ones_st")
            nc.gpsimd.memset(ones_st[:], 1.0)
            for h in range(2):
                vv = vaug[h][:].rearrange("p (k e) -> p k e", e=33)
                nc.vector.tensor_copy(out=vv[:, :, 32], in_=ones_st[:])

            bankc = 0  # global rotating-slot cursor

            # ---- qkv builds -------------------------------------------------
            # q/k stage g covers FOUR itiles (4g..4g+4) stacked in one
            # [128,1024] psum pair: rows 64*s + 32*h + d, cols 512*c2 hold
            # itile 4g+2s+c2.  Then per (s,h): one [32,1024] fp8 hi-copy and
            # one [32,1024] lo-subtract straight into the DoubleRow operand
            # tiles (partition-shifted engine ops; no staging, no fold DMAs):
            #   qSide[h]: rows 0:32 = hi, 32:64 = lo; t dim duplicated by DMA
            #   kSide[h]: t=0 = hi, t=1 = lo; rows 32:64 duplicated by DMA
            cpeng = [nc.scalar, nc.vector, nc.scalar, nc.gpsimd]
            sbeng = [nc.vector, nc.gpsimd, nc.vector, nc.vector]

            def qk_stage(w_sb, side, g):
                nonlocal bankc
                rts = [rhalf(bankc)[0], rhalf(bankc + 1)[0]]
                bankc += 2
                for s in range(2):
                    for c2 in range(2):
                        it = 4 * g + 2 * s + c2
                        for cc in range(2):
                            nc.tensor.matmul(
                                rts[c2][64 * s:64 * (s + 1), :],
                                lhsT=w_sb[:, cc, :],
                                rhs=xT[cc][:, 512 * it:512 * (it + 1)],
                                start=(cc == 0), stop=(cc == 1),
                                tile_position=(0, 64 * s),
                                skip_group_check=True,
                            )
                for s in range(2):
                    for c2 in range(2):
                        it = 4 * g + 2 * s + c2
                        cols = slice(512 * it, 512 * (it + 1))
                        for h in range(2):
                            r0 = 64 * s + 32 * h
                            if side is qSide:
                                hi_ap = side[h][0:32, 0, cols]
                                lo_ap = side[h][32:64, 0, cols]
                            else:
                                hi_ap = side[h][0:32, 0, cols]
                                lo_ap = side[h][0:32, 1, cols]
                            eng = cpeng[2 * s + h]
                            if eng is nc.scalar:
                                eng.copy(out=hi_ap, in_=rts[c2][r0:r0 + 32, :])
                            else:
                                eng.tensor_copy(out=hi_ap, in_=rts[c2][r0:r0 + 32, :])
                            sbeng[2 * s + h].tensor_tensor(
                                out=lo_ap, in0=rts[c2][r0:r0 + 32, :], in1=hi_ap,
                                op=ALU.subtract,
                            )

            def dup_group(g):
                # duplicate this 2048-col group's qSide t dim / kSide rows
                cols = slice(2048 * g, 2048 * (g + 1))
                for h in range(2):
                    nc.sync.dma_start(out=qSide[h][:, 1, cols],
                                      in_=qSide[h][:, 0, cols])
                    nc.sync.dma_start(out=kSide[h][32:64, :, cols],
                                      in_=kSide[h][0:32, :, cols])

            def v_round(k0):
                nonlocal bankc
                rt2, _ = rhalf(bankc)
                bankc += 1
                for k in range(k0, k0 + 4):
                    for cc in range(2):
                        nc.tensor.matmul(
                            rt2[:, 64 * (k - k0):64 * (k - k0) + 64],
                            lhsT=xT[cc][:, P * k:P * (k + 1)],
                            rhs=wv_sb[:, cc, :],
                            start=(cc == 0), stop=(cc == 1),
                        )
                sv2 = rt2[:, 0:256].rearrange("p (k d) -> p k d", d=64)
                for h in range(2):
                    vv = vaug[h][:].rearrange("p (k e) -> p k e", e=33)
                    nc.vector.tensor_copy(
                        out=vv[:, k0:k0 + 4, 0:32],
                        in_=sv2[:, :, 32 * h:32 * (h + 1)],
                    )

            # prologue: all of q/k/v for BOTH heads (x DMA pieces feed in
            # column order; stage g needs cols 2048g:2048(g+1))
            for g in range(2):
                qk_stage(wk_sb, kSide, g)
                v_round(16 * g)
                v_round(16 * g + 4)
                qk_stage(wq_sb, qSide, g)
                v_round(16 * g + 8)
                v_round(16 * g + 12)
                dup_group(g)

            # ================= attention chunk stream ======================
            def chunk_meta(c):
                h = c // (ITILES * NCH)
                it = (c // NCH) % ITILES
                j = c % NCH
                return h, it, j

            NC_TOT = 2 * ITILES * NCH
            pattern = _make_pattern(NC_TOT, EXP_W)

            slot_of = {}

            def emit_sim(c):
                nonlocal bankc
                h, it, j = chunk_meta(c)
                slot_of[c] = bankc
                rt_, _ = rhalf(bankc)
                bankc += 1
                nc.tensor.matmul(
                    rt_[:, 0:512],
                    lhsT=kSide[h][:, :, P * j:P * (j + 1)],
                    rhs=qSide[h][:, :, 512 * it:512 * (it + 1)],
                    start=True, stop=True,
                    perf_mode=DR,
                )

            def emit_exp(c):
                # quantum = one chunk -> one rot slot tile, one eslab tile
                rt_, _ = rhalf(slot_of[c])
                es_ap = eslabs[c % EROT][:]
                rt_ap = rt_[:, 0:512]
                eng = pattern[c % len(pattern)]
                if eng == "A":
                    nc.scalar.activation(es_ap, rt_ap, AF.Exp, scale=SCALE)
                elif eng == "V":
                    nc.vector.tensor_scalar(
                        out=es_ap.bitcast(I16), in0=rt_ap,
                        scalar1=SCH_A, scalar2=SCH_B,
                        op0=ALU.mult, op1=ALU.add,
                    )
                else:
                    nc.gpsimd.tensor_scalar(
                        out=es_ap.bitcast(I16), in0=rt_ap,
                        scalar1=SCH_A, scalar2=SCH_B,
                        op0=ALU.mult, op1=ALU.add,
                    )

            def emit_av(c):
                h, it, j = chunk_meta(c)
                es = eslabs[c % EROT]
                for ic in range(4):
                    nc.tensor.matmul(
                        avp[:, 33 * ic:33 * ic + 33],
                        lhsT=es[:, 128 * ic:128 * (ic + 1)],
                        rhs=vaug[h][:, 33 * j:33 * j + 33],
                        start=(j == 0 and ic == 0), stop=(j == NCH - 1),
                        skip_group_check=True,
                    )

            def emit_itile_stage(h, it):
                dv = avp[:, 0:132].rearrange("p (ic e) -> p ic e", e=33)[:, :, 32]
                r0 = 32 * h + 4 * it
                nc.vector.reciprocal(out=rden[:, r0:r0 + 4], in_=dv)
                for ic in range(4):
                    nc.gpsimd.tensor_scalar_mul(
                        av_sc[:, 32 * ic:32 * (ic + 1)],
                        avp[:, 33 * ic:33 * ic + 32],
                        rden[:, r0 + ic:r0 + ic + 1],
                    )

            def emit_itile_transpose(h, it):
                nonlocal bankc
                rt_, _ = rhalf(bankc)
                bankc += 1
                for ic in range(4):
                    nc.tensor.transpose(
                        rt_[0:32, 64 * ic:64 * (ic + 1)].bitcast(BF),
                        av_sc[:, 32 * ic:32 * (ic + 1)],
                        identb[:],
                    )
                nc.vector.tensor_copy(
                    out=outT[32 * h:32 * (h + 1), 512 * it:512 * (it + 1)],
                    in_=rt_[0:32, 0:256].bitcast(BF),
                )

            def emit_y(it):
                for ic in range(4):
                    k = 4 * it + ic
                    cols = slice(256 * (ic % 2), 256 * (ic % 2) + C)
                    nc.tensor.matmul(
                        tb[:, cols],
                        lhsT=outT[:, P * k:P * (k + 1)],
                        rhs=wo_sb[:],
                        start=True, stop=True,
                        tile_position=(0, 0),
                    )
                    yo = ytmpp.tile([P, C], FP, tag="yo")
                    nc.scalar.copy(out=yo[:], in_=tb[:, cols])
                    nc.sync.dma_start(out=y_d[P * k:P * (k + 1), :], in_=yo[:])

            hooks = {}

            def add_hook(c, fn):
                hooks.setdefault(min(c, NC_TOT - 1), []).append(fn)

            for c in range(NC_TOT):
                emit_sim(c)
                emit_exp(c)
                if c >= AV_LAG:
                    # av of c-AV_LAG MUST precede the exp quantum closing at c:
                    # that exp overwrites the eslab cols av(c-AV_LAG) reads.
                    emit_av(c - AV_LAG)
                    ch, cit, cj = chunk_meta(c - AV_LAG)
                    if cj == NCH - 1:
                        emit_itile_stage(ch, cit)
                        add_hook(c + 10, lambda ch=ch, cit=cit:
                                 emit_itile_transpose(ch, cit))
                        if ch == 1 and cit > 0:
                            add_hook(c + 14, lambda cit=cit: emit_y(cit - 1))
                for fn in hooks.pop(c, ()):
                    fn()

            # tail: remaining avs, last i-tile stage/transpose, last y projs
            for c in range(NC_TOT - AV_LAG, NC_TOT):
                emit_av(c)
                ch, cit, cj = chunk_meta(c)
                if cj == NCH - 1:
                    emit_itile_stage(ch, cit)
                    emit_itile_transpose(ch, cit)
            for fn_list in [hooks[k] for k in sorted(hooks)]:
                for fn in fn_list:
                    fn()
            emit_y(ITILES - 2)
            emit_y(ITILES - 1)

    _split_excess_waits(nc, mybir)
    return nc


def _split_excess_waits(nc, mybir, maxw=1, carrier_cap=1):
    """Hoist excess semaphore waits onto InstEventSemaphore carriers."""
    skip = {
        "InstEventSemaphore", "InstCall",
        "InstUnconditionalBranch", "InstISA", "InstRegisterMove",
    }
    for f in nc.m.functions:
        for blk in f.blocks:
            idx = 0
            while idx < len(blk.instructions):
                ins = blk.instructions[idx]
                si = getattr(ins, "sync_info", None)
                if (
                    si is not None and si.on_wait and len(si.on_wait) > maxw
                    and type(ins).__name__ not in skip
                ):
                    waits = list(si.on_wait)
                    keep, excess = waits[:maxw], waits[maxw:]
                    at = idx
                    if (at > 0 and type(blk.instructions[at - 1]).__name__
                            == "InstLdweights"):
                        at -= 1
                    n_ins = 0
                    for i in range(0, len(excess), carrier_cap):
                        ev = mybir.InstEventSemaphore(
                            name=nc.get_next_instruction_name(),
                            engine=ins.engine,
                            ins=[], outs=[],
                            sync_info=mybir.SyncInfo(
                                on_wait=excess[i:i + carrier_cap], on_update=[]
                            ),
                        )
                        nc.register_instruction(ev)
                        blk.instructions.insert(at + n_ins, ev)
                        n_ins += 1
                    ins.sync_info = mybir.SyncInfo(
                        on_wait=keep, on_update=list(si.on_update or [])
                    )
                    idx += n_ins
                idx += 1
    return nc


def get_nc():
    if "nc" not in _CACHED:
        _CACHED["nc"] = _build_nc()
    return _CACHED["nc"]


def make_in_maps(x, w_qkv, w_out):
    """Host-side sharding: core c -> batch c//2, heads (c%2)*2, (c%2)*2+1."""
    import ml_dtypes
    B = x.shape[0]
    xf = np.ascontiguousarray(x.reshape(B, N, C))
    in_maps = []
    for core in range(8):
        b, hp = core // 2, core % 2
        h0, h1 = 2 * hp, 2 * hp + 1
        wq = np.concatenate(
            [w_qkv[:, h * DH:(h + 1) * DH] for h in (h0, h1)], axis=1
        )
        wk = np.concatenate(
            [w_qkv[:, 128 + h * DH: 128 + (h + 1) * DH] for h in (h0, h1)], axis=1
        )
        wv = np.concatenate(
            [w_qkv[:, 256 + h * DH: 256 + (h + 1) * DH] for h in (h0, h1)], axis=1
        )
        wo = np.concatenate(
            [w_out[h * DH:(h + 1) * DH, :] for h in (h0, h1)], axis=0
        )
        in_maps.append({
            "xt": np.ascontiguousarray(xf[b].T.astype(ml_dtypes.bfloat16)).view(np.uint16),
            "wq": np.ascontiguousarray(wq.astype(np.float32)),
            "wk": np.ascontiguousarray(wk.astype(np.float32)),
            "wv": np.ascontiguousarray(wv.astype(np.float32)),
            "wo": np.ascontiguousarray(wo.astype(np.float32)),
        })
    return in_maps


def kernel(x, w_qkv, w_out, b_out):
    from concourse.bass_utils import run_bass_kernel_spmd

    nc = get_nc()
    in_maps = make_in_maps(
        np.asarray(x, dtype=np.float32),
        np.asarray(w_qkv, dtype=np.float32),
        np.asarray(w_out, dtype=np.float32),
    )
    res = run_bass_kernel_spmd(nc, in_maps, list(range(8))).results
    B, H, W = 4, 64, 64
    y = np.empty((B, N, C), dtype=np.float32)
    for b in range(B):
        y[b] = res[2 * b]["y"] + res[2 * b + 1]["y"]
    y += np.asarray(b_out, dtype=np.float32)
    return y.reshape(B, H, W, C)


# revision 12
# speedup vs baseline: 1.4871x; 1.1324x over previous
"""Trainium2 Bass kernel v3 for spatial self-attention (nn_Attention_90615220011343).

Per-core (core c -> batch c//2, heads 2*(c%2), 2*(c%2)+1):
    qkv = x @ w_qkv; per head sim^T[j,i] = k^T q; attn = softmax; out = attn@v
    y_partial = sum_h (out_h/den) @ wo_h ; host sums head-pairs + bias.

v3 changes vs v2 (cost-model driven):
  - sim matmul in fp8e4m3 with MatmulPerfMode.DoubleRow: 0.5 cycles/row
    (vs 1.0 for bf16/fp32r) -> 256 PE cycles per [128,512] chunk.
    Precision recovered by error compensation: q = qhi + qlo, k = khi + klo
    (hi = fp8(x), lo = fp8(x - hi)); the 128 DoubleRow contraction slots
    (64 partitions x 2) hold all four cross products (qhi+qlo)x(khi+klo),
    so the product is exact up to the ~0.1% lo-rounding. attn scale (1/sqrt(32))
    is folded into the exp input scaling, not into q.
  - exp runs on THREE engines: ACT (exact, activation Exp with scale),
    DVE + Pool (Schraudolph int16 bitcast = bf16 exp approx). Pattern
    weighted by engine rates (ACT 0.83, DVE 1.04, Pool 1.39 ns/row).
  - q^T/k^T builds write 2-itile/panel stacks into one [128,512] psum
    ([64*s + 32*h + d] partitions), so the fp8 hi-copy + lo-subtract are
    [128,512] ops (4x fewer engine rows); SBUF->SBUF DMAs (cheap issue from
    the gpsimd ring) fold the stacks into the DoubleRow operand layouts:
      qSide[h]: [64, 2, N] rows = (qhi d | qlo d), t duplicated
      kSide[h]: [64, 2, N] cols t = (khi | klo), rows duplicated
  - transposes of the normalized attention output are bf16 (1 c/r, vs fp32r
    4 c/r when free<256); outT copy reads bf16 psum (DVE 2x_1p mode).
"""

import numpy as np

HEADS = 4
DH = 32
N = 4096
C = 256
P = 128
NCH = 32          # j-chunks of 128 tokens
ITILES = 8        # i tiles of 512
ROT = 6           # rotating psum banks for sim chunks
EROT = 12         # eslab rotation depth (chunks)
AV_LAG = 10       # chunks between sim emission and its av matmuls
SCALE = float(DH ** -0.5)
# bf16 Schraudolph exp: round(s*a+b) as int16 IS bf16(exp(s)) up to ~3%
# sawtooth; softmax normalization cancels most. a absorbs the attn scale.
SCH_A = float(2 ** 7 / np.log(2)) * SCALE
SCH_B = float(127 * 2 ** 7) - 7.6
# exp engine pattern per chunk (1 quantum = 1 chunk = [128,512] psum slot):
# weighted by measured engine service times (ACT 612, DVE 658, Pool ~560 ns)
EXP_W = {"A": 9, "V": 9, "P": 14}

_CACHED = {}


def _make_pattern(total, weights):
    acc = {k: 0.0 for k in weights}
    wsum = float(sum(weights.values()))
    out = []
    for _ in range(total):
        for k in acc:
            acc[k] += weights[k]
        kbest = max(acc, key=lambda kk: (acc[kk], kk))
        acc[kbest] -= wsum
        out.append(kbest)
    return out


def _build_nc():
    import concourse.bass as bass
    import concourse.mybir as mybir
    from concourse.tile import TileContext
    from concourse.masks import make_identity

    FP = mybir.dt.float32
    BF = mybir.dt.bfloat16
    E4 = mybir.dt.float8e4
    U16 = mybir.dt.uint16
    I16 = mybir.dt.int16
    AF = mybir.ActivationFunctionType
    ALU = mybir.AluOpType
    DR = mybir.MatmulPerfMode.DoubleRow

    nc = bass.Bass(target_bir_lowering=False)
    xt_d = nc.declare_dram_parameter("xt", [C, N], U16, isOutput=False)
    wq_d = nc.declare_dram_parameter("wq", [C, 64], FP, isOutput=False)
    wk_d = nc.declare_dram_parameter("wk", [C, 64], FP, isOutput=False)
    wv_d = nc.declare_dram_parameter("wv", [C, 64], FP, isOutput=False)
    wo_d = nc.declare_dram_parameter("wo", [64, C], FP, isOutput=False)
    y_d = nc.declare_dram_parameter("y", [N, C], FP, isOutput=True)

    with TileContext(nc) as tc:
        with (
            tc.tile_pool(name="const", bufs=1) as constp,
            tc.tile_pool(name="big", bufs=1) as bigp,
            tc.tile_pool(name="ytmp", bufs=4) as ytmpp,
            tc.tile_pool(name="psR", bufs=1, space="PSUM") as psR,
            tc.tile_pool(name="psV", bufs=1, space="PSUM") as psV,
            tc.tile_pool(name="psT", bufs=1, space="PSUM") as psT,
        ):
            ident = constp.tile([P, P], FP, tag="ident")
            make_identity(nc, ident[:])
            identb = constp.tile([P, P], BF, tag="identb")
            nc.vector.tensor_copy(out=identb[:], in_=ident[:])

            # ---- persistent SBUF ----
            xT = [bigp.tile([P, N], BF, tag=f"xT{cc}", name=f"xT{cc}") for cc in range(2)]
            qSide = [bigp.tile([64, 2, N], E4, tag=f"qS{h}", name=f"qS{h}")
                     for h in range(2)]
            kSide = [bigp.tile([64, 2, N], E4, tag=f"kS{h}", name=f"kS{h}")
                     for h in range(2)]
            vaug = [bigp.tile([P, 33 * NCH], BF, tag=f"vaug{h}", name=f"vaug{h}")
                    for h in range(2)]
            outT = bigp.tile([64, N], BF, tag="outT")
            rden = bigp.tile([P, 64], FP, tag="rden")
            av_sc = bigp.tile([P, P], BF, tag="av_sc")
            eslabs = [bigp.tile([P, 512], BF, tag=f"esl{t}", name=f"esl{t}")
                      for t in range(EROT)]

            wq_sb = bigp.tile([P, 2, 64], BF, tag="wq")
            wk_sb = bigp.tile([P, 2, 64], BF, tag="wk")
            wv_sb = bigp.tile([P, 2, 64], BF, tag="wv")
            wo_sb = bigp.tile([64, C], BF, tag="wo")

            # ---- psum ----
            rots = [psR.tile([P, 512], FP, tag=f"R{t}", name=f"rotT{t}")
                    for t in range(ROT)]
            avp = psV.tile([P, 512], FP, tag="V")      # cols 0:132 in use
            tb = psT.tile([P, 512], FP, tag="T")       # y projections

            def rhalf(bc):
                return rots[bc % ROT], 0

            # ---- weight loads + conversion ----
            wq_st = bigp.tile([P, 2, 64], FP, tag="wq_st")
            wk_st = bigp.tile([P, 2, 64], FP, tag="wk_st")
            wv_st = bigp.tile([P, 2, 64], FP, tag="wv_st")
            wo_st = bigp.tile([64, C], FP, tag="wo_st")
            for cc in range(2):
                nc.sync.dma_start(out=wq_st[:, cc, :], in_=wq_d[cc * P:(cc + 1) * P, :])
                nc.sync.dma_start(out=wk_st[:, cc, :], in_=wk_d[cc * P:(cc + 1) * P, :])
                nc.sync.dma_start(out=wv_st[:, cc, :], in_=wv_d[cc * P:(cc + 1) * P, :])
            nc.sync.dma_start(out=wo_st[:], in_=wo_d[:])
            nc.vector.tensor_copy(out=wq_sb[:], in_=wq_st[:])
            nc.vector.tensor_copy(out=wk_sb[:], in_=wk_st[:])
            nc.vector.tensor_copy(out=wv_sb[:], in_=wv_st[:])
            nc.vector.tensor_copy(out=wo_sb[:], in_=wo_st[:])

            # ---- x load (pre-transposed bf16 from host), 3 DMA rings ----
            dma_engines = [nc.sync, nc.scalar, nc.gpsimd]
            for s in range(4):
                for cc in range(2):
                    dma_engines[(2 * s + cc) % 3].dma_start(
                        out=xT[cc][:, 1024 * s:1024 * (s + 1)].bitcast(U16),
                        in_=xt_d[P * cc:P * (cc + 1),
                                 1024 * s:1024 * (s + 1)],
                    )

            ones_st = bigp.tile([P, NCH], BF, tag="ones_st")
            nc.gpsimd.memset(ones_st[:], 1.0)
            for h in range(2):
                vv = vaug[h][:].rearrange("p (k e) -> p k e", e=33)
                nc.vector.tensor_copy(out=vv[:, :, 32], in_=ones_st[:])

            bankc = 0  # global rotating-slot cursor

            # ---- qkv builds -------------------------------------------------
            # q/k stage g covers FOUR itiles (4g..4g+4) stacked in one
            # [128,1024] psum pair: rows 64*s + 32*h + d, cols 512*c2 hold
            # itile 4g+2s+c2.  Then per (s,h): one [32,1024] fp8 hi-copy and
            # one [32,1024] lo-subtract straight into the DoubleRow operand
            # tiles (partition-shifted engine ops; no staging, no fold DMAs):
            #   qSide[h]: rows 0:32 = hi, 32:64 = lo; t dim duplicated by DMA
            #   kSide[h]: t=0 = hi, t=1 = lo; rows 32:64 duplicated by DMA
            cpeng = [nc.scalar, nc.gpsimd, nc.scalar, nc.gpsimd]
            sbeng = [nc.vector, nc.gpsimd, nc.vector, nc.gpsimd]

            def qk_stage(w_sb, side, g):
                nonlocal bankc
                rts = [rhalf(bankc)[0], rhalf(bankc + 1)[0]]
                bankc += 2
                for s in range(2):
                    for c2 in range(2):
                        it = 4 * g + 2 * s + c2
                        for cc in range(2):
                            nc.tensor.matmul(
                                rts[c2][64 * s:64 * (s + 1), :],
                                lhsT=w_sb[:, cc, :],
                                rhs=xT[cc][:, 512 * it:512 * (it + 1)],
                                start=(cc == 0), stop=(cc == 1),
                                tile_position=(0, 64 * s),
                                skip_group_check=True,
                            )
                for s in range(2):
                    for c2 in range(2):
                        it = 4 * g + 2 * s + c2
                        cols = slice(512 * it, 512 * (it + 1))
                        for h in range(2):
                            r0 = 64 * s + 32 * h
                            if side is qSide:
                                hi_ap = side[h][0:32, 0, cols]
                                lo_ap = side[h][32:64, 0, cols]
                            else:
                                hi_ap = side[h][0:32, 0, cols]
                                lo_ap = side[h][0:32, 1, cols]
                            eng = cpeng[2 * s + h]
                            if eng is nc.scalar:
                                eng.copy(out=hi_ap, in_=rts[c2][r0:r0 + 32, :])
                            else:
                                eng.tensor_copy(out=hi_ap, in_=rts[c2][r0:r0 + 32, :])
                            sbeng[2 * s + h].tensor_tensor(
                                out=lo_ap, in0=rts[c2][r0:r0 + 32, :], in1=hi_ap,
                                op=ALU.subtract,
                            )

            def dup_group(g):
                # duplicate this 2048-col group's qSide t dim / kSide rows
                cols = slice(2048 * g, 2048 * (g + 1))
                for h in range(2):
                    nc.sync.dma_start(out=qSide[h][:, 1, cols],
                                      in_=qSide[h][:, 0, cols])
                    nc.sync.dma_start(out=kSide[h][32:64, :, cols],
                                      in_=kSide[h][0:32, :, cols])

            def v_round(k0):
                nonlocal bankc
                rt2, _ = rhalf(bankc)
                bankc += 1
                for k in range(k0, k0 + 4):
                    for cc in range(2):
                        nc.tensor.matmul(
                            rt2[:, 64 * (k - k0):64 * (k - k0) + 64],
                            lhsT=xT[cc][:, P * k:P * (k + 1)],
                            rhs=wv_sb[:, cc, :],
                            start=(cc == 0), stop=(cc == 1),
                        )
                sv2 = rt2[:, 0:256].rearrange("p (k d) -> p k d", d=64)
                for h in range(2):
                    vv = vaug[h][:].rearrange("p (k e) -> p k e", e=33)
                    nc.gpsimd.tensor_copy(
                        out=vv[:, k0:k0 + 4, 0:32],
                        in_=sv2[:, :, 32 * h:32 * (h + 1)],
                    )

            # prologue: all of q/k/v for BOTH heads (x DMA pieces feed in
            # column order; stage g needs cols 2048g:2048(g+1))
            for g in range(2):
                qk_stage(wk_sb, kSide, g)
                v_round(16 * g)
                v_round(16 * g + 4)
                qk_stage(wq_sb, qSide, g)
                v_round(16 * g + 8)
                v_round(16 * g + 12)
                dup_group(g)

            # ================= attention chunk stream ======================
            def chunk_meta(c):
                h = c // (ITILES * NCH)
                it = (c // NCH) % ITILES
                j = c % NCH
                return h, it, j

            NC_TOT = 2 * ITILES * NCH
            pattern = _make_pattern(NC_TOT, EXP_W)

            slot_of = {}

            def emit_sim(c):
                nonlocal bankc
                h, it, j = chunk_meta(c)
                slot_of[c] = bankc
                rt_, _ = rhalf(bankc)
                bankc += 1
                nc.tensor.matmul(
                    rt_[:, 0:512],
                    lhsT=kSide[h][:, :, P * j:P * (j + 1)],
                    rhs=qSide[h][:, :, 512 * it:512 * (it + 1)],
                    start=True, stop=True,
                    perf_mode=DR,
                )

            def emit_exp(c):
                # quantum = one chunk -> one rot slot tile, one eslab tile
                rt_, _ = rhalf(slot_of[c])
                es_ap = eslabs[c % EROT][:]
                rt_ap = rt_[:, 0:512]
                eng = pattern[c % len(pattern)]
                if eng == "A":
                    nc.scalar.activation(es_ap, rt_ap, AF.Exp, scale=SCALE)
                elif eng == "V":
                    nc.vector.tensor_scalar(
                        out=es_ap.bitcast(I16), in0=rt_ap,
                        scalar1=SCH_A, scalar2=SCH_B,
                        op0=ALU.mult, op1=ALU.add,
                    )
                else:
                    nc.gpsimd.tensor_scalar(
                        out=es_ap.bitcast(I16), in0=rt_ap,
                        scalar1=SCH_A, scalar2=SCH_B,
                        op0=ALU.mult, op1=ALU.add,
                    )

            def emit_av(c):
                h, it, j = chunk_meta(c)
                es = eslabs[c % EROT]
                for ic in range(4):
                    nc.tensor.matmul(
                        avp[:, 33 * ic:33 * ic + 33],
                        lhsT=es[:, 128 * ic:128 * (ic + 1)],
                        rhs=vaug[h][:, 33 * j:33 * j + 33],
                        start=(j == 0 and ic == 0), stop=(j == NCH - 1),
                        skip_group_check=True,
                    )

            def emit_itile_stage(h, it):
                dv = avp[:, 0:132].rearrange("p (ic e) -> p ic e", e=33)[:, :, 32]
                r0 = 32 * h + 4 * it
                nc.vector.reciprocal(out=rden[:, r0:r0 + 4], in_=dv)
                for ic in range(4):
                    nc.gpsimd.tensor_scalar_mul(
                        av_sc[:, 32 * ic:32 * (ic + 1)],
                        avp[:, 33 * ic:33 * ic + 32],
                        rden[:, r0 + ic:r0 + ic + 1],
                    )

            def emit_itile_transpose(h, it):
                nonlocal bankc
                rt_, _ = rhalf(bankc)
                bankc += 1
                for ic in range(4):
                    nc.tensor.transpose(
                        rt_[0:32, 64 * ic:64 * (ic + 1)].bitcast(BF),
                        av_sc[:, 32 * ic:32 * (ic + 1)],
                        identb[:],
                    )
                nc.vector.tensor_copy(
                    out=outT[32 * h:32 * (h + 1), 512 * it:512 * (it + 1)],
                    in_=rt_[0:32, 0:256].bitcast(BF),
                )

            def emit_y(it):
                for ic in range(4):
                    k = 4 * it + ic
                    cols = slice(256 * (ic % 2), 256 * (ic % 2) + C)
                    nc.tensor.matmul(
                        tb[:, cols],
                        lhsT=outT[:, P * k:P * (k + 1)],
                        rhs=wo_sb[:],
                        start=True, stop=True,
                        tile_position=(0, 0),
                    )
                    yo = ytmpp.tile([P, C], FP, tag="yo")
                    nc.scalar.copy(out=yo[:], in_=tb[:, cols])
                    nc.sync.dma_start(out=y_d[P * k:P * (k + 1), :], in_=yo[:])

            hooks = {}

            def add_hook(c, fn):
                hooks.setdefault(min(c, NC_TOT - 1), []).append(fn)

            for c in range(NC_TOT):
                emit_sim(c)
                emit_exp(c)
                if c >= AV_LAG:
                    # av of c-AV_LAG MUST precede the exp quantum closing at c:
                    # that exp overwrites the eslab cols av(c-AV_LAG) reads.
                    emit_av(c - AV_LAG)
                    ch, cit, cj = chunk_meta(c - AV_LAG)
                    if cj == NCH - 1:
                        emit_itile_stage(ch, cit)
                        add_hook(c + 10, lambda ch=ch, cit=cit:
                                 emit_itile_transpose(ch, cit))
                        if ch == 1 and cit > 0:
                            add_hook(c + 14, lambda cit=cit: emit_y(cit - 1))
                for fn in hooks.pop(c, ()):
                    fn()

            # tail: remaining avs, last i-tile stage/transpose, last y projs
            for c in range(NC_TOT - AV_LAG, NC_TOT):
                emit_av(c)
                ch, cit, cj = chunk_meta(c)
                if cj == NCH - 1:
                    emit_itile_stage(ch, cit)
                    emit_itile_transpose(ch, cit)
            for fn_list in [hooks[k] for k in sorted(hooks)]:
                for fn in fn_list:
                    fn()
            emit_y(ITILES - 2)
            emit_y(ITILES - 1)

    _split_excess_waits(nc, mybir)
    return nc


def _split_excess_waits(nc, mybir, maxw=1, carrier_cap=1):
    """Hoist excess semaphore waits onto InstEventSemaphore carriers."""
    skip = {
        "InstEventSemaphore", "InstCall",
        "InstUnconditionalBranch", "InstISA", "InstRegisterMove",
    }
    for f in nc.m.functions:
        for blk in f.blocks:
            idx = 0
            while idx < len(blk.instructions):
                ins = blk.instructions[idx]
                si = getattr(ins, "sync_info", None)
                if (
                    si is not None and si.on_wait and len(si.on_wait) > maxw
                    and type(ins).__name__ not in skip
                ):
                    waits = list(si.on_wait)
                    keep, excess = waits[:maxw], waits[maxw:]
                    at = idx
                    if (at > 0 and type(blk.instructions[at - 1]).__name__
                            == "InstLdweights"):
                        at -= 1
                    n_ins = 0
                    for i in range(0, len(excess), carrier_cap):
                        ev = mybir.InstEventSemaphore(
                            name=nc.get_next_instruction_name(),
                            engine=ins.engine,
                            ins=[], outs=[],
                            sync_info=mybir.SyncInfo(
                                on_wait=excess[i:i + carrier_cap], on_update=[]
                            ),
                        )
                        nc.register_instruction(ev)
                        blk.instructions.insert(at + n_ins, ev)
                        n_ins += 1
                    ins.sync_info = mybir.SyncInfo(
                        on_wait=keep, on_update=list(si.on_update or [])
                    )
                    idx += n_ins
                idx += 1
    return nc


def get_nc():
    if "nc" not in _CACHED:
        _CACHED["nc"] = _build_nc()
    return _CACHED["nc"]


def make_in_maps(x, w_qkv, w_out):
    """Host-side sharding: core c -> batch c//2, heads (c%2)*2, (c%2)*2+1."""
    import ml_dtypes
    B = x.shape[0]
    xf = np.ascontiguousarray(x.reshape(B, N, C))
    in_maps = []
    for core in range(8):
        b, hp = core // 2, core % 2
        h0, h1 = 2 * hp, 2 * hp + 1
        wq = np.concatenate(
            [w_qkv[:, h * DH:(h + 1) * DH] for h in (h0, h1)], axis=1
        )
        wk = np.concatenate(
            [w_qkv[:, 128 + h * DH: 128 + (h + 1) * DH] for h in (h0, h1)], axis=1
        )
        wv = np.concatenate(
            [w_qkv[:, 256 + h * DH: 256 + (h + 1) * DH] for h in (h0, h1)], axis=1
        )
        wo = np.concatenate(
            [w_out[h * DH:(h + 1) * DH, :] for h in (h0, h1)], axis=0
        )
        in_maps.append({
            "xt": np.ascontiguousarray(xf[b].T.astype(ml_dtypes.bfloat16)).view(np.uint16),
            "wq": np.ascontiguousarray(wq.astype(np.float32)),
            "wk": np.ascontiguousarray(wk.astype(np.float32)),
            "wv": np.ascontiguousarray(wv.astype(np.float32)),
            "wo": np.ascontiguousarray(wo.astype(np.float32)),
        })
    return in_maps


def kernel(x, w_qkv, w_out, b_out):
    from concourse.bass_utils import run_bass_kernel_spmd

    nc = get_nc()
    in_maps = make_in_maps(
        np.asarray(x, dtype=np.float32),
        np.asarray(w_qkv, dtype=np.float32),
        np.asarray(w_out, dtype=np.float32),
    )
    res = run_bass_kernel_spmd(nc, in_maps, list(range(8))).results
    B, H, W = 4, 64, 64
    y = np.empty((B, N, C), dtype=np.float32)
    for b in range(B):
        y[b] = res[2 * b]["y"] + res[2 * b + 1]["y"]
    y += np.asarray(b_out, dtype=np.float32)
    return y.reshape(B, H, W, C)


# revision 13
# speedup vs baseline: 1.5076x; 1.0138x over previous
"""Trainium2 Bass kernel v3 for spatial self-attention (nn_Attention_90615220011343).

Per-core (core c -> batch c//2, heads 2*(c%2), 2*(c%2)+1):
    qkv = x @ w_qkv; per head sim^T[j,i] = k^T q; attn = softmax; out = attn@v
    y_partial = sum_h (out_h/den) @ wo_h ; host sums head-pairs + bias.

v3 changes vs v2 (cost-model driven):
  - sim matmul in fp8e4m3 with MatmulPerfMode.DoubleRow: 0.5 cycles/row
    (vs 1.0 for bf16/fp32r) -> 256 PE cycles per [128,512] chunk.
    Precision recovered by error compensation: q = qhi + qlo, k = khi + klo
    (hi = fp8(x), lo = fp8(x - hi)); the 128 DoubleRow contraction slots
    (64 partitions x 2) hold all four cross products (qhi+qlo)x(khi+klo),
    so the product is exact up to the ~0.1% lo-rounding. attn scale (1/sqrt(32))
    is folded into the exp input scaling, not into q.
  - exp runs on THREE engines: ACT (exact, activation Exp with scale),
    DVE + Pool (Schraudolph int16 bitcast = bf16 exp approx). Pattern
    weighted by engine rates (ACT 0.83, DVE 1.04, Pool 1.39 ns/row).
  - q^T/k^T builds write 2-itile/panel stacks into one [128,512] psum
    ([64*s + 32*h + d] partitions), so the fp8 hi-copy + lo-subtract are
    [128,512] ops (4x fewer engine rows); SBUF->SBUF DMAs (cheap issue from
    the gpsimd ring) fold the stacks into the DoubleRow operand layouts:
      qSide[h]: [64, 2, N] rows = (qhi d | qlo d), t duplicated
      kSide[h]: [64, 2, N] cols t = (khi | klo), rows duplicated
  - transposes of the normalized attention output are bf16 (1 c/r, vs fp32r
    4 c/r when free<256); outT copy reads bf16 psum (DVE 2x_1p mode).
"""

import numpy as np

HEADS = 4
DH = 32
N = 4096
C = 256
P = 128
NCH = 32          # j-chunks of 128 tokens
ITILES = 8        # i tiles of 512
ROT = 6           # rotating psum banks for sim chunks
EROT = 12         # eslab rotation depth (chunks)
AV_LAG = 10       # chunks between sim emission and its av matmuls
SCALE = float(DH ** -0.5)
# bf16 Schraudolph exp: round(s*a+b) as int16 IS bf16(exp(s)) up to ~3%
# sawtooth; softmax normalization cancels most. a absorbs the attn scale.
SCH_A = float(2 ** 7 / np.log(2)) * SCALE
SCH_B = float(127 * 2 ** 7) - 7.6
# exp engine pattern per chunk (1 quantum = 1 chunk = [128,512] psum slot):
# weighted by measured engine service times (ACT 612, DVE 658, Pool ~560 ns)
EXP_W = {"A": 19, "V": 17, "P": 28}

_CACHED = {}


def _make_pattern(total, weights):
    acc = {k: 0.0 for k in weights}
    wsum = float(sum(weights.values()))
    out = []
    for _ in range(total):
        for k in acc:
            acc[k] += weights[k]
        kbest = max(acc, key=lambda kk: (acc[kk], kk))
        acc[kbest] -= wsum
        out.append(kbest)
    return out


def _build_nc():
    import concourse.bass as bass
    import concourse.mybir as mybir
    from concourse.tile import TileContext
    from concourse.masks import make_identity

    FP = mybir.dt.float32
    BF = mybir.dt.bfloat16
    E4 = mybir.dt.float8e4
    U16 = mybir.dt.uint16
    I16 = mybir.dt.int16
    AF = mybir.ActivationFunctionType
    ALU = mybir.AluOpType
    DR = mybir.MatmulPerfMode.DoubleRow

    nc = bass.Bass(target_bir_lowering=False)
    xt_d = nc.declare_dram_parameter("xt", [C, N], U16, isOutput=False)
    wq_d = nc.declare_dram_parameter("wq", [C, 64], FP, isOutput=False)
    wk_d = nc.declare_dram_parameter("wk", [C, 64], FP, isOutput=False)
    wv_d = nc.declare_dram_parameter("wv", [C, 64], FP, isOutput=False)
    wo_d = nc.declare_dram_parameter("wo", [64, C], FP, isOutput=False)
    y_d = nc.declare_dram_parameter("y", [N, C], FP, isOutput=True)

    with TileContext(nc) as tc:
        with (
            tc.tile_pool(name="const", bufs=1) as constp,
            tc.tile_pool(name="big", bufs=1) as bigp,
            tc.tile_pool(name="ytmp", bufs=4) as ytmpp,
            tc.tile_pool(name="psR", bufs=1, space="PSUM") as psR,
            tc.tile_pool(name="psV", bufs=1, space="PSUM") as psV,
            tc.tile_pool(name="psT", bufs=1, space="PSUM") as psT,
        ):
            ident = constp.tile([P, P], FP, tag="ident")
            make_identity(nc, ident[:])
            identb = constp.tile([P, P], BF, tag="identb")
            nc.vector.tensor_copy(out=identb[:], in_=ident[:])

            # ---- persistent SBUF ----
            xT = [bigp.tile([P, N], BF, tag=f"xT{cc}", name=f"xT{cc}") for cc in range(2)]
            qSide = [bigp.tile([64, 2, N], E4, tag=f"qS{h}", name=f"qS{h}")
                     for h in range(2)]
            kSide = [bigp.tile([64, 2, N], E4, tag=f"kS{h}", name=f"kS{h}")
                     for h in range(2)]
            vaug = [bigp.tile([P, 33 * NCH], BF, tag=f"vaug{h}", name=f"vaug{h}")
                    for h in range(2)]
            outT = bigp.tile([64, N], BF, tag="outT")
            rden = bigp.tile([P, 64], FP, tag="rden")
            av_sc = bigp.tile([P, P], BF, tag="av_sc")
            eslabs = [bigp.tile([P, 512], BF, tag=f"esl{t}", name=f"esl{t}")
                      for t in range(EROT)]

            wq_sb = bigp.tile([P, 2, 64], BF, tag="wq")
            wk_sb = bigp.tile([P, 2, 64], BF, tag="wk")
            wv_sb = bigp.tile([P, 2, 64], BF, tag="wv")
            wo_sb = bigp.tile([64, C], BF, tag="wo")

            # ---- psum ----
            rots = [psR.tile([P, 512], FP, tag=f"R{t}", name=f"rotT{t}")
                    for t in range(ROT)]
            avp = psV.tile([P, 512], FP, tag="V")      # cols 0:132 in use
            tb = psT.tile([P, 512], FP, tag="T")       # y projections

            def rhalf(bc):
                return rots[bc % ROT], 0

            # ---- weight loads + conversion ----
            wq_st = bigp.tile([P, 2, 64], FP, tag="wq_st")
            wk_st = bigp.tile([P, 2, 64], FP, tag="wk_st")
            wv_st = bigp.tile([P, 2, 64], FP, tag="wv_st")
            wo_st = bigp.tile([64, C], FP, tag="wo_st")
            for cc in range(2):
                nc.sync.dma_start(out=wq_st[:, cc, :], in_=wq_d[cc * P:(cc + 1) * P, :])
                nc.sync.dma_start(out=wk_st[:, cc, :], in_=wk_d[cc * P:(cc + 1) * P, :])
                nc.sync.dma_start(out=wv_st[:, cc, :], in_=wv_d[cc * P:(cc + 1) * P, :])
            nc.sync.dma_start(out=wo_st[:], in_=wo_d[:])
            nc.vector.tensor_copy(out=wq_sb[:], in_=wq_st[:])
            nc.vector.tensor_copy(out=wk_sb[:], in_=wk_st[:])
            nc.vector.tensor_copy(out=wv_sb[:], in_=wv_st[:])
            nc.vector.tensor_copy(out=wo_sb[:], in_=wo_st[:])

            # ---- x load (pre-transposed bf16 from host) ----
            # scalar+gpsimd rings; SP stays free for weight/dup/y DMAs
            dma_engines = [nc.scalar, nc.gpsimd]
            for s in range(8):
                for cc in range(2):
                    dma_engines[(2 * s + cc) % 2].dma_start(
                        out=xT[cc][:, 512 * s:512 * (s + 1)].bitcast(U16),
                        in_=xt_d[P * cc:P * (cc + 1),
                                 512 * s:512 * (s + 1)],
                    )

            ones_st = bigp.tile([P, NCH], BF, tag="ones_st")
            nc.gpsimd.memset(ones_st[:], 1.0)
            for h in range(2):
                vv = vaug[h][:].rearrange("p (k e) -> p k e", e=33)
                nc.vector.tensor_copy(out=vv[:, :, 32], in_=ones_st[:])

            bankc = 0  # global rotating-slot cursor

            # ---- qkv builds -------------------------------------------------
            # q/k stage g covers FOUR itiles (4g..4g+4) stacked in one
            # [128,1024] psum pair: rows 64*s + 32*h + d, cols 512*c2 hold
            # itile 4g+2s+c2.  Then per (s,h): one [32,1024] fp8 hi-copy and
            # one [32,1024] lo-subtract straight into the DoubleRow operand
            # tiles (partition-shifted engine ops; no staging, no fold DMAs):
            #   qSide[h]: rows 0:32 = hi, 32:64 = lo; t dim duplicated by DMA
            #   kSide[h]: t=0 = hi, t=1 = lo; rows 32:64 duplicated by DMA
            cpeng = [nc.scalar, nc.gpsimd, nc.scalar, nc.gpsimd]
            sbeng = [nc.vector, nc.gpsimd, nc.vector, nc.gpsimd]

            def qk_stage(w_sb, side, g):
                nonlocal bankc
                rts = [rhalf(bankc)[0], rhalf(bankc + 1)[0]]
                bankc += 2
                for s in range(2):
                    for c2 in range(2):
                        it = 4 * g + 2 * s + c2
                        for cc in range(2):
                            nc.tensor.matmul(
                                rts[c2][64 * s:64 * (s + 1), :],
                                lhsT=w_sb[:, cc, :],
                                rhs=xT[cc][:, 512 * it:512 * (it + 1)],
                                start=(cc == 0), stop=(cc == 1),
                                tile_position=(0, 64 * s),
                                skip_group_check=True,
                            )
                for s in range(2):
                    for c2 in range(2):
                        it = 4 * g + 2 * s + c2
                        cols = slice(512 * it, 512 * (it + 1))
                        for h in range(2):
                            r0 = 64 * s + 32 * h
                            if side is qSide:
                                hi_ap = side[h][0:32, 0, cols]
                                lo_ap = side[h][32:64, 0, cols]
                            else:
                                hi_ap = side[h][0:32, 0, cols]
                                lo_ap = side[h][0:32, 1, cols]
                            eng = cpeng[2 * s + h]
                            if eng is nc.scalar:
                                eng.copy(out=hi_ap, in_=rts[c2][r0:r0 + 32, :])
                            else:
                                eng.tensor_copy(out=hi_ap, in_=rts[c2][r0:r0 + 32, :])
                            sbeng[2 * s + h].tensor_tensor(
                                out=lo_ap, in0=rts[c2][r0:r0 + 32, :], in1=hi_ap,
                                op=ALU.subtract,
                            )

            def dup_group(g):
                # duplicate this 2048-col group's qSide t dim / kSide rows
                cols = slice(2048 * g, 2048 * (g + 1))
                for h in range(2):
                    nc.sync.dma_start(out=qSide[h][:, 1, cols],
                                      in_=qSide[h][:, 0, cols])
                    nc.sync.dma_start(out=kSide[h][32:64, :, cols],
                                      in_=kSide[h][0:32, :, cols])

            def v_round(k0):
                nonlocal bankc
                rt2, _ = rhalf(bankc)
                bankc += 1
                for k in range(k0, k0 + 4):
                    for cc in range(2):
                        nc.tensor.matmul(
                            rt2[:, 64 * (k - k0):64 * (k - k0) + 64],
                            lhsT=xT[cc][:, P * k:P * (k + 1)],
                            rhs=wv_sb[:, cc, :],
                            start=(cc == 0), stop=(cc == 1),
                        )
                sv2 = rt2[:, 0:256].rearrange("p (k d) -> p k d", d=64)
                for h in range(2):
                    vv = vaug[h][:].rearrange("p (k e) -> p k e", e=33)
                    nc.gpsimd.tensor_copy(
                        out=vv[:, k0:k0 + 4, 0:32],
                        in_=sv2[:, :, 32 * h:32 * (h + 1)],
                    )

            # prologue: all of q/k/v for BOTH heads (x DMA pieces feed in
            # column order; stage g needs cols 2048g:2048(g+1))
            for g in range(2):
                qk_stage(wk_sb, kSide, g)
                v_round(16 * g)
                v_round(16 * g + 4)
                qk_stage(wq_sb, qSide, g)
                v_round(16 * g + 8)
                v_round(16 * g + 12)
                dup_group(g)

            # ================= attention chunk stream ======================
            def chunk_meta(c):
                h = c // (ITILES * NCH)
                it = (c // NCH) % ITILES
                j = c % NCH
                return h, it, j

            NC_TOT = 2 * ITILES * NCH
            pattern = _make_pattern(NC_TOT, EXP_W)

            slot_of = {}

            def emit_sim(c):
                nonlocal bankc
                h, it, j = chunk_meta(c)
                slot_of[c] = bankc
                rt_, _ = rhalf(bankc)
                bankc += 1
                nc.tensor.matmul(
                    rt_[:, 0:512],
                    lhsT=kSide[h][:, :, P * j:P * (j + 1)],
                    rhs=qSide[h][:, :, 512 * it:512 * (it + 1)],
                    start=True, stop=True,
                    perf_mode=DR,
                )

            def emit_exp(c):
                # quantum = one chunk -> one rot slot tile, one eslab tile
                rt_, _ = rhalf(slot_of[c])
                es_ap = eslabs[c % EROT][:]
                rt_ap = rt_[:, 0:512]
                eng = pattern[c % len(pattern)]
                if eng == "A":
                    nc.scalar.activation(es_ap, rt_ap, AF.Exp, scale=SCALE)
                elif eng == "V":
                    nc.vector.tensor_scalar(
                        out=es_ap.bitcast(I16), in0=rt_ap,
                        scalar1=SCH_A, scalar2=SCH_B,
                        op0=ALU.mult, op1=ALU.add,
                    )
                else:
                    nc.gpsimd.tensor_scalar(
                        out=es_ap.bitcast(I16), in0=rt_ap,
                        scalar1=SCH_A, scalar2=SCH_B,
                        op0=ALU.mult, op1=ALU.add,
                    )

            def emit_av(c):
                h, it, j = chunk_meta(c)
                es = eslabs[c % EROT]
                for ic in range(4):
                    nc.tensor.matmul(
                        avp[:, 33 * ic:33 * ic + 33],
                        lhsT=es[:, 128 * ic:128 * (ic + 1)],
                        rhs=vaug[h][:, 33 * j:33 * j + 33],
                        start=(j == 0 and ic == 0), stop=(j == NCH - 1),
                        skip_group_check=True,
                    )

            def emit_itile_stage(h, it):
                dv = avp[:, 0:132].rearrange("p (ic e) -> p ic e", e=33)[:, :, 32]
                r0 = 32 * h + 4 * it
                nc.vector.reciprocal(out=rden[:, r0:r0 + 4], in_=dv)
                for ic in range(4):
                    nc.gpsimd.tensor_scalar_mul(
                        av_sc[:, 32 * ic:32 * (ic + 1)],
                        avp[:, 33 * ic:33 * ic + 32],
                        rden[:, r0 + ic:r0 + ic + 1],
                    )

            def emit_itile_transpose(h, it):
                nonlocal bankc
                rt_, _ = rhalf(bankc)
                bankc += 1
                for ic in range(4):
                    nc.tensor.transpose(
                        rt_[0:32, 64 * ic:64 * (ic + 1)].bitcast(BF),
                        av_sc[:, 32 * ic:32 * (ic + 1)],
                        identb[:],
                    )
                nc.vector.tensor_copy(
                    out=outT[32 * h:32 * (h + 1), 512 * it:512 * (it + 1)],
                    in_=rt_[0:32, 0:256].bitcast(BF),
                )

            def emit_y(it):
                for ic in range(4):
                    k = 4 * it + ic
                    cols = slice(256 * (ic % 2), 256 * (ic % 2) + C)
                    nc.tensor.matmul(
                        tb[:, cols],
                        lhsT=outT[:, P * k:P * (k + 1)],
                        rhs=wo_sb[:],
                        start=True, stop=True,
                        tile_position=(0, 0),
                    )
                    yo = ytmpp.tile([P, C], FP, tag="yo")
                    nc.scalar.copy(out=yo[:], in_=tb[:, cols])
                    nc.sync.dma_start(out=y_d[P * k:P * (k + 1), :], in_=yo[:])

            hooks = {}

            def add_hook(c, fn):
                hooks.setdefault(min(c, NC_TOT - 1), []).append(fn)

            for c in range(NC_TOT):
                emit_sim(c)
                if c >= AV_LAG:
                    # av of c-AV_LAG MUST precede the exp emitted at c: that
                    # exp overwrites the eslab tile av(c-AV_LAG) reads when
                    # AV_LAG == EROT - (c mod ...); keep av first.
                    emit_av(c - AV_LAG)
                emit_exp(c)
                if c >= AV_LAG:
                    ch, cit, cj = chunk_meta(c - AV_LAG)
                    if cj == NCH - 1:
                        emit_itile_stage(ch, cit)
                        add_hook(c + 10, lambda ch=ch, cit=cit:
                                 emit_itile_transpose(ch, cit))
                        if ch == 1 and cit > 0:
                            add_hook(c + 14, lambda cit=cit: emit_y(cit - 1))
                for fn in hooks.pop(c, ()):
                    fn()

            # tail: remaining avs, last i-tile stage/transpose, last y projs
            for c in range(NC_TOT - AV_LAG, NC_TOT):
                emit_av(c)
                ch, cit, cj = chunk_meta(c)
                if cj == NCH - 1:
                    emit_itile_stage(ch, cit)
                    emit_itile_transpose(ch, cit)
            for fn_list in [hooks[k] for k in sorted(hooks)]:
                for fn in fn_list:
                    fn()
            emit_y(ITILES - 2)
            emit_y(ITILES - 1)

    _split_excess_waits(nc, mybir)
    return nc


def _split_excess_waits(nc, mybir, maxw=1, carrier_cap=1):
    """Hoist excess semaphore waits onto InstEventSemaphore carriers."""
    skip = {
        "InstEventSemaphore", "InstCall",
        "InstUnconditionalBranch", "InstISA", "InstRegisterMove",
    }
    for f in nc.m.functions:
        for blk in f.blocks:
            idx = 0
            while idx < len(blk.instructions):
                ins = blk.instructions[idx]
                si = getattr(ins, "sync_info", None)
                if (
                    si is not None and si.on_wait and len(si.on_wait) > maxw
                    and type(ins).__name__ not in skip
                ):
                    waits = list(si.on_wait)
                    keep, excess = waits[:maxw], waits[maxw:]
                    at = idx
                    if (at > 0 and type(blk.instructions[at - 1]).__name__
                            == "InstLdweights"):
                        at -= 1
                    n_ins = 0
                    for i in range(0, len(excess), carrier_cap):
                        ev = mybir.InstEventSemaphore(
                            name=nc.get_next_instruction_name(),
                            engine=ins.engine,
                            ins=[], outs=[],
                            sync_info=mybir.SyncInfo(
                                on_wait=excess[i:i + carrier_cap], on_update=[]
                            ),
                        )
                        nc.register_instruction(ev)
                        blk.instructions.insert(at + n_ins, ev)
                        n_ins += 1
                    ins.sync_info = mybir.SyncInfo(
                        on_wait=keep, on_update=list(si.on_update or [])
                    )
                    idx += n_ins
                idx += 1
    return nc


def get_nc():
    if "nc" not in _CACHED:
        _CACHED["nc"] = _build_nc()
    return _CACHED["nc"]


def make_in_maps(x, w_qkv, w_out):
    """Host-side sharding: core c -> batch c//2, heads (c%2)*2, (c%2)*2+1."""
    import ml_dtypes
    B = x.shape[0]
    xf = np.ascontiguousarray(x.reshape(B, N, C))
    in_maps = []
    for core in range(8):
        b, hp = core // 2, core % 2
        h0, h1 = 2 * hp, 2 * hp + 1
        wq = np.concatenate(
            [w_qkv[:, h * DH:(h + 1) * DH] for h in (h0, h1)], axis=1
        )
        wk = np.concatenate(
            [w_qkv[:, 128 + h * DH: 128 + (h + 1) * DH] for h in (h0, h1)], axis=1
        )
        wv = np.concatenate(
            [w_qkv[:, 256 + h * DH: 256 + (h + 1) * DH] for h in (h0, h1)], axis=1
        )
        wo = np.concatenate(
            [w_out[h * DH:(h + 1) * DH, :] for h in (h0, h1)], axis=0
        )
        in_maps.append({
            "xt": np.ascontiguousarray(xf[b].T.astype(ml_dtypes.bfloat16)).view(np.uint16),
            "wq": np.ascontiguousarray(wq.astype(np.float32)),
            "wk": np.ascontiguousarray(wk.astype(np.float32)),
            "wv": np.ascontiguousarray(wv.astype(np.float32)),
            "wo": np.ascontiguousarray(wo.astype(np.float32)),
        })
    return in_maps


def kernel(x, w_qkv, w_out, b_out):
    from concourse.bass_utils import run_bass_kernel_spmd

    nc = get_nc()
    in_maps = make_in_maps(
        np.asarray(x, dtype=np.float32),
        np.asarray(w_qkv, dtype=np.float32),
        np.asarray(w_out, dtype=np.float32),
    )
    res = run_bass_kernel_spmd(nc, in_maps, list(range(8))).results
    B, H, W = 4, 64, 64
    y = np.empty((B, N, C), dtype=np.float32)
    for b in range(B):
        y[b] = res[2 * b]["y"] + res[2 * b + 1]["y"]
    y += np.asarray(b_out, dtype=np.float32)
    return y.reshape(B, H, W, C)
